# revision 9
# baseline (speedup 1.0000x reference)
"""MeshUpdateNet (EdgeConv message passing + MLP decoder) on 8 Trainium2
NeuronCores via Bass/Tile.

Strategy (no collectives; sharding by destination node):

  - Nodes are sharded by destination: sort nodes by degree (desc) and deal
    them round-robin to the 8 cores. Each core owns NC = N/8 nodes and all
    edges pointing at them (~E/8 per core, balanced), and its local node
    list is degree-sorted.
  - Edges are laid out rank-major: rank r holds the r-th edge of every
    local node with deg > r. Because nodes are degree-sorted, rank r's
    slots form a prefix [0, N_r) of the local node axis, so segment-max
    becomes a sequence of elementwise max ops over aligned prefixes - no
    scatter, no segmented reduce.
  - The host pre-gathers the per-slot features [xi ; xj] into a [6, L]
    bf16 stream per core. The round-robin deal makes the rank widths
    common across cores (+-1, padded by duplicating an existing edge of
    the node - max is idempotent so duplicates are free), so one SPMD
    program serves all 8.
  - Device per core (per 1024-slot tile):
      mm1: K=6 matmul (w1m6^T s) -> ps1          [PE]
      h1 = relu(ps1 + b1) -> bf16                [ACT]
      mm2: K=128 matmul (w2^T h1) -> ps2         [PE]
      agg = max(agg, ps2 + b2)                   [DVE scalar_tensor_tensor]
    every DRAIN_EVERY-th tile instead drains ps2 on ACT
    (t2 = relu(ps2 + b2) -> bf16) and does a cheap 2x-mode bf16 max on
    DVE, balancing the ACT/DVE load (DVE's fused fp32-from-PSUM op is
    the more expensive of the two).
  - relu-before-max: since relu(max(x)) == max(relu(x)) and agg is
    initialized to 0 (>= 0 always), the max chain accumulates
    relu(h2 + b2) for free, so the tail consumes agg directly as the
    (bf16) input of the encoder matmul - no separate relu pass.
  - Tail per 512-node tile: enc = w3^T agg (b3 folded into b4'),
    r5 = relu(w4^T enc + b4'), dec-matmul w5 packed 4 tiles per PSUM
    group via tile_position col groups, tanh(+b5) -> [99, 512] fp32,
    DMA'd out as 4 dense [3, 512] strips. pos + 0.1*tanh is applied on
    the host during unpacking.
  - Nodes with no edges would aggregate garbage from pad slots; they are
    patched on the host with the closed-form constant output (their row
    is independent of x). With E/N = 16 there are essentially none.
"""
import sys

sys.path.insert(0, '/opt/trn_rl_repo')

import numpy as np
import ml_dtypes

import concourse.bass as bass
import concourse.tile as tile
from concourse import bacc, mybir
from concourse import bass_utils

F32 = mybir.dt.float32
BF16 = mybir.dt.bfloat16
BF = ml_dtypes.bfloat16

N_CORES = 8
TILE_W = 1024      # edge tile width (2 psum banks)
MM_W = 512         # max matmul moving free dim (1 psum bank)
CHUNK = 2048       # stream DMA chunk (cols); ring of N_BUFS buffers
N_BUFS = 8         # stream ring depth
NODE_W = 512       # tail node-tile width
GROUP = 4          # node tiles packed per psum group in the tail
WARMUP_MM = 14     # gapless matmul chain to ramp the PE p-state
DRAIN_EVERY = 6    # every Nth edge tile drains ps2 on ACT instead of DVE


def make_schedule(deg, n_nodes):
    """Common (all-cores) edge/tail tiling from the global degree array."""
    nodes_sorted = np.argsort(-deg, kind='stable')
    deg_sorted = deg[nodes_sorted]
    d_max = int(deg_sorted[0]) if len(deg_sorted) else 0
    M = np.searchsorted(-deg_sorted, -(np.arange(d_max) + 1), side='right')
    NC = n_nodes // N_CORES
    N_r = -(-M // N_CORES)              # ceil(M_r/8): common rank width
    T_r = -(-N_r // TILE_W)
    L = int((T_r * TILE_W).sum())
    rank_off = np.zeros(d_max + 1, np.int64)
    np.cumsum(T_r * TILE_W, out=rank_off[1:])
    etiles = []
    for r in range(d_max):
        w_left = int(N_r[r])
        for t in range(int(T_r[r])):
            w = min(TILE_W, w_left - t * TILE_W)
            etiles.append((int(rank_off[r]) + t * TILE_W, t * TILE_W, w))
    n_ntiles = -(-NC // NODE_W)
    n_groups = -(-n_ntiles // GROUP)
    return dict(nodes_sorted=nodes_sorted, deg_sorted=deg_sorted, d_max=d_max,
                NC=NC, N_r=N_r, T_r=T_r, L=L, rank_off=rank_off, etiles=etiles,
                n_ntiles=n_ntiles, n_groups=n_groups)


def build_nc(sched):
    NC, L = sched['NC'], sched['L']
    etiles = sched['etiles']
    n_ntiles, n_groups = sched['n_ntiles'], sched['n_groups']
    GPC = n_groups * NODE_W

    nc = bacc.Bacc("TRN2", target_bir_lowering=False, debug=False,
                   enable_asserts=False, num_devices=N_CORES)

    xs_d = nc.dram_tensor("xs", [6, L], BF16, kind="ExternalInput").ap()
    w1m_d = nc.dram_tensor("w1m", [128, 128], BF16, kind="ExternalInput").ap()
    w2_d = nc.dram_tensor("w2", [128, 128], BF16, kind="ExternalInput").ap()
    w3_d = nc.dram_tensor("w3", [128, 128], BF16, kind="ExternalInput").ap()
    w4_d = nc.dram_tensor("w4", [128, 128], BF16, kind="ExternalInput").ap()
    w5_d = nc.dram_tensor("w5", [128, 3], BF16, kind="ExternalInput").ap()
    b1_d = nc.dram_tensor("b1", [128, 1], F32, kind="ExternalInput").ap()
    b2_d = nc.dram_tensor("b2", [128, 1], F32, kind="ExternalInput").ap()
    b4p_d = nc.dram_tensor("b4p", [128, 1], F32, kind="ExternalInput").ap()
    b5pk_d = nc.dram_tensor("b5pk", [99, 1], F32, kind="ExternalInput").ap()
    out_d = nc.dram_tensor("outpk", [12, GPC], F32, kind="ExternalOutput").ap()

    RELU = mybir.ActivationFunctionType.Relu
    TANH = mybir.ActivationFunctionType.Tanh
    COPY = mybir.ActivationFunctionType.Copy
    ADD = mybir.AluOpType.add
    MAX = mybir.AluOpType.max

    with tile.TileContext(nc) as tc:
        with (
            tc.tile_pool(name="const", bufs=1) as cp,
            tc.tile_pool(name="aggp", bufs=1) as aggp,
            tc.tile_pool(name="stream", bufs=1) as sp,
            tc.tile_pool(name="work", bufs=4) as wp,
        ):
            # constants needed early
            w2_s = cp.tile([128, 128], BF16)
            nc.sync.dma_start(w2_s[:], w2_d[:])
            w1m_s = cp.tile([128, 128], BF16)
            nc.sync.dma_start(w1m_s[:], w1m_d[:])
            b1_s = cp.tile([128, 1], F32)
            nc.sync.dma_start(b1_s[:], b1_d[:])
            b2_s = cp.tile([128, 1], F32)
            nc.sync.dma_start(b2_s[:], b2_d[:])

            # PE warm-up needs this first on the vector queue
            warm_rhs = wp.tile([128, 512], BF16, tag="warmrhs")
            nc.vector.memset(warm_rhs[:], 0.0)

            # Stream chunk ring [128, CHUNK] x N_BUFS: rows 0-5 carry the
            # DMA'd [xi;xj] stream; rows 6-127 are zeroed once so mm1
            # contracts over K=128 with a zero-padded w1m. K=6 matmuls
            # starve the PE_HAM activity monitor (only 6 of 128 rows
            # active) and pin the PE at its 1.2 GHz throttled state -
            # measured 454us of K=4/8 with K=6, vs warm at K=128.
            # Small buffers keep each memset ~1.7us so the first chunk
            # DMA lands before the warm-up chain ends; a startup PE gap
            # re-throttles the HAM and a throttled stream never recovers
            # (measured 78-82us stuck at K=4/8).
            n_chunks = -(-L // CHUNK)
            ch_bufs = []
            for bi in range(N_BUFS):
                chb = sp.tile([128, CHUNK], BF16, tag=f"xs{bi}")
                ch_bufs.append(chb)
                if bi < N_BUFS // 2:
                    nc.vector.memset(chb[:], 0.0)
                else:
                    nc.gpsimd.memset(chb[:], 0.0)

            # agg (bf16, init 0): relu-before-max makes 0 the identity.
            # Split the memset so the first columns are ready when the
            # first max lands (~17us).
            agg = aggp.tile([128, NC], BF16)
            A_SPLIT = min(4096, NC)
            nc.vector.memset(agg[:, :A_SPLIT], 0.0)
            if A_SPLIT < NC:
                nc.gpsimd.memset(agg[:, A_SPLIT:], 0.0)

            chunk_tiles = {}
            for ci in range(n_chunks):
                cw = min(CHUNK, L - ci * CHUNK)
                ch = ch_bufs[ci % N_BUFS]
                nc.sync.dma_start(ch[:6, :cw],
                                  xs_d[:, ci * CHUNK: ci * CHUNK + cw])
                chunk_tiles[ci] = ch

            # PE warm-up: gapless back-to-back matmul chain in its own
            # psum scope; the p-state ramp needs >3us of uninterrupted PE
            # execution.
            with tc.tile_pool(name="psW", bufs=4, space="PSUM") as pW:
                for i in range(WARMUP_MM):
                    wps = pW.tile([128, 512], F32, tag="warm")
                    nc.tensor.matmul(wps[:], w2_s[:], warm_rhs[:],
                                     start=True, stop=True)

            with (
                tc.tile_pool(name="psA", bufs=2, space="PSUM") as pA,
                tc.tile_pool(name="psB", bufs=2, space="PSUM") as pB,
            ):
                for ti, (so, c0, W) in enumerate(etiles):
                    ci, off = so // CHUNK, so % CHUNK
                    ch = chunk_tiles[ci]
                    ps1 = pA.tile([128, TILE_W], F32, tag="p1")
                    for h in range(0, W, MM_W):
                        w = min(MM_W, W - h)
                        nc.tensor.matmul(ps1[:, h:h + w], w1m_s[:],
                                         ch[:, off + h: off + h + w],
                                         start=True, stop=True)
                    h1 = wp.tile([128, TILE_W], BF16, tag="h1")
                    nc.scalar.activation(h1[:, :W], ps1[:, :W], RELU,
                                         bias=b1_s[:, 0:1])
                    ps2 = pB.tile([128, TILE_W], F32, tag="p2")
                    for h in range(0, W, MM_W):
                        w = min(MM_W, W - h)
                        nc.tensor.matmul(ps2[:, h:h + w], w2_s[:],
                                         h1[:, h:h + w], start=True, stop=True)
                    if ti % DRAIN_EVERY == DRAIN_EVERY - 1:
                        # balance: drain on ACT, cheap bf16 2x max on DVE
                        t2 = wp.tile([128, TILE_W], BF16, tag="t2")
                        nc.scalar.activation(t2[:, :W], ps2[:, :W], RELU,
                                             bias=b2_s[:, 0:1])
                        nc.vector.tensor_tensor(
                            out=agg[:, c0:c0 + W], in0=t2[:, :W],
                            in1=agg[:, c0:c0 + W], op=MAX)
                    else:
                        # fused add-b2 + max straight from PSUM
                        nc.vector.scalar_tensor_tensor(
                            out=agg[:, c0:c0 + W], in0=ps2[:, :W],
                            scalar=b2_s[:, 0:1], in1=agg[:, c0:c0 + W],
                            op0=ADD, op1=MAX)

            # tail constants (issued late so they don't delay the stream)
            w3_s = cp.tile([128, 128], BF16)
            nc.sync.dma_start(w3_s[:], w3_d[:])
            w4_s = cp.tile([128, 128], BF16)
            nc.sync.dma_start(w4_s[:], w4_d[:])
            w5_s = cp.tile([128, 3], BF16)
            nc.sync.dma_start(w5_s[:], w5_d[:])
            b4p_s = cp.tile([128, 1], F32)
            nc.sync.dma_start(b4p_s[:], b4p_d[:])
            b5pk_s = cp.tile([99, 1], F32)
            nc.sync.dma_start(b5pk_s[:], b5pk_d[:])

            with (
                tc.tile_pool(name="psT", bufs=2, space="PSUM") as pT,
                tc.tile_pool(name="psG", bufs=2, space="PSUM") as pG,
            ):
                for g in range(n_groups):
                    ps5 = pG.tile([99, NODE_W], F32, tag="p5")
                    for j in range(GROUP):
                        t = g * GROUP + j
                        if t >= n_ntiles:
                            # mm5 below only writes rows it owns; zero the
                            # missing strips so the out DMA reads zeros
                            if g == n_groups - 1:
                                nc.vector.memset(ps5[32 * j:32 * j + 3, :], 0.0)
                            continue
                        c0 = t * NODE_W
                        W = min(NODE_W, NC - c0)
                        # enc (sans b3): agg is already relu(max(...)+b2)
                        ps3 = pT.tile([128, NODE_W], F32, tag="p3")
                        nc.tensor.matmul(ps3[:, :W], w3_s[:], agg[:, c0:c0 + W],
                                         start=True, stop=True)
                        e4 = wp.tile([128, NODE_W], BF16, tag="e4")
                        if j % 2 == 0:
                            nc.scalar.activation(e4[:, :W], ps3[:, :W], COPY)
                        else:
                            nc.vector.tensor_copy(e4[:, :W], ps3[:, :W])
                        ps4 = pT.tile([128, NODE_W], F32, tag="p4")
                        nc.tensor.matmul(ps4[:, :W], w4_s[:], e4[:, :W],
                                         start=True, stop=True)
                        r5 = wp.tile([128, NODE_W], BF16, tag="r5")
                        if j % 2 == 0:
                            nc.vector.tensor_scalar(
                                out=r5[:, :W], in0=ps4[:, :W],
                                scalar1=b4p_s[:, 0:1], scalar2=0.0,
                                op0=ADD, op1=MAX)
                        else:
                            nc.scalar.activation(r5[:, :W], ps4[:, :W], RELU,
                                                 bias=b4p_s[:, 0:1])
                        if W < NODE_W:
                            nc.vector.memset(ps5[32 * j:32 * j + 3, W:], 0.0)
                        nc.tensor.matmul(ps5[32 * j:32 * j + 3, :W], w5_s[:],
                                         r5[:, :W], start=True, stop=True,
                                         tile_position=(0, 32 * j))
                    s_t = wp.tile([99, NODE_W], F32, tag="s")
                    nc.scalar.activation(s_t[:], ps5[:], TANH,
                                         bias=b5pk_s[:, 0:1])
                    gc = g * NODE_W
                    for j in range(GROUP):
                        nc.sync.dma_start(out_d[3 * j:3 * j + 3, gc:gc + NODE_W],
                                          s_t[32 * j:32 * j + 3, :])
    nc.compile()
    return nc


def make_inputs(x, pos, w1, b1, w2, b2, w3, b3, w4, b4, w5, b5,
                src, dst, sched):
    n_nodes = x.shape[0]
    E = src.shape[0]
    L, d_max = sched['L'], sched['d_max']
    N_r, rank_off = sched['N_r'], sched['rank_off']
    nodes_sorted = sched['nodes_sorted']

    order = np.argsort(dst, kind='stable')
    src_sorted = src[order]
    deg = np.bincount(dst, minlength=n_nodes)
    starts = np.zeros(n_nodes + 1, np.int64)
    np.cumsum(deg, out=starts[1:])

    # msg @ w1 = [xi ; xj-xi] @ w1 = [xi ; xj] @ [[w1a-w1b]; [w1b]]
    w1a, w1b = w1[:3], w1[3:]
    w1m = np.zeros((128, 128), np.float32)
    w1m[:6] = np.vstack([w1a - w1b, w1b])
    w1m = w1m.astype(BF)
    b4p = (b3 @ w4 + b4).astype(np.float32).reshape(128, 1)   # fold b3
    b5pk = np.zeros((99, 1), np.float32)
    for j in range(GROUP):
        b5pk[32 * j:32 * j + 3, 0] = b5

    common = dict(
        w1m=w1m, w2=w2.astype(BF), w3=w3.astype(BF), w4=w4.astype(BF),
        w5=w5.astype(BF), b1=b1.reshape(128, 1).astype(np.float32),
        b2=b2.reshape(128, 1).astype(np.float32), b4p=b4p, b5pk=b5pk)

    slot_pos = np.zeros(L, np.int64)
    for r in range(d_max):
        w = int(N_r[r])
        o = int(rank_off[r])
        slot_pos[o:o + w] = np.arange(w)

    in_maps = []
    for c in range(N_CORES):
        loc_nodes = nodes_sorted[c::N_CORES]
        loc_deg = deg[loc_nodes]
        loc_start = starts[loc_nodes]
        slot_src = np.zeros(L, np.int64)
        for r in range(d_max):
            w = int(N_r[r])
            o = int(rank_off[r])
            has = loc_deg[:w] > r
            # pad slots duplicate the node's first edge (max-idempotent);
            # deg-0 nodes gather garbage and are patched on the host
            idx = np.where(has, loc_start[:w] + r, loc_start[:w])
            np.minimum(idx, E - 1, out=idx)
            slot_src[o:o + w] = src_sorted[idx]
        xi_loc = x[loc_nodes]
        xs = np.empty((6, L), BF)
        xs[0:3] = xi_loc[slot_pos].T.astype(BF)
        xs[3:6] = x[slot_src].T.astype(BF)
        in_maps.append(dict(xs=xs, **common))
    return in_maps


def unpack_outputs(results, sched, pos, deg, w3, b3, w4, b4, w5, b5):
    NC = sched['NC']
    nodes_sorted = sched['nodes_sorted']
    n_groups = sched['n_groups']
    n = len(nodes_sorted)
    out_full = np.zeros((n, 3), np.float32)
    for c in range(N_CORES):
        outpk = results[c]['outpk'].reshape(12, n_groups, NODE_W)
        tiles = np.zeros((3, n_groups * GROUP, NODE_W), np.float32)
        for j in range(GROUP):
            tiles[:, j::GROUP, :] = outpk[3 * j:3 * j + 3]
        tanh_t = tiles.reshape(3, -1)[:, :NC]
        loc = nodes_sorted[c::N_CORES]
        out_full[loc] = pos[loc] + 0.1 * tanh_t.T
    deg0 = deg == 0
    if deg0.any():
        # closed form for isolated nodes: agg = 0 -> enc = b3
        enc0 = b3
        dec0 = np.maximum(enc0 @ w4 + b4, 0.0) @ w5 + b5
        out_full[deg0] = pos[deg0] + 0.1 * np.tanh(dec0)
    return out_full


def run(inputs, trace=False, tmpdir=None):
    x = np.asarray(inputs['x'], np.float32)
    pos = np.asarray(inputs['pos'], np.float32)
    ei = np.asarray(inputs['edge_index'])
    src = ei[0].astype(np.int64)
    dst = ei[1].astype(np.int64)
    deg = np.bincount(dst, minlength=x.shape[0])
    sched = make_schedule(deg, x.shape[0])
    nc = build_nc(sched)
    args = [np.asarray(inputs[k], np.float32) for k in
            ('w1', 'b1', 'w2', 'b2', 'w3', 'b3', 'w4', 'b4', 'w5', 'b5')]
    in_maps = make_inputs(x, pos, *args, src, dst, sched)
    res = bass_utils.run_bass_kernel_spmd(
        nc, in_maps, core_ids=list(range(N_CORES)), trace=trace, tmpdir=tmpdir)
    w3_, b3_, w4_, b4_, w5_, b5_ = args[4:]
    out = unpack_outputs(res.results, sched, pos, deg,
                         w3_, b3_, w4_, b4_, w5_, b5_)
    return out, res


def kernel(**inputs):
    out, _ = run(inputs, trace=False)
    return out


# revision 14
# speedup vs baseline: 1.2467x; 1.2467x over previous
"""MeshUpdateNet (EdgeConv message passing + MLP decoder) on 8 Trainium2
NeuronCores via Bass/Tile.

Strategy (no collectives; sharding by destination node):

  - Nodes are sharded by destination: sort nodes by degree (desc) and deal
    them round-robin to the 8 cores. Each core owns NC = N/8 nodes and all
    edges pointing at them (~E/8 per core, balanced), and its local node
    list is degree-sorted.
  - Edges are laid out rank-major: rank r holds the r-th edge of every
    local node with deg > r. Because nodes are degree-sorted, rank r's
    slots form a prefix [0, N_r) of the local node axis, so segment-max
    becomes a sequence of elementwise max ops over aligned prefixes - no
    scatter, no segmented reduce.
  - The host pre-gathers the per-slot features [xi ; xj] into a [6, L]
    bf16 stream per core. The round-robin deal makes the rank widths
    common across cores (+-1, padded by duplicating an existing edge of
    the node - max is idempotent so duplicates are free), so one SPMD
    program serves all 8.
  - Device per core (per 1024-slot tile):
      mm1: K=6 matmul (w1m6^T s) -> ps1          [PE]
      h1 = relu(ps1 + b1) -> bf16                [ACT]
      mm2: K=128 matmul (w2^T h1) -> ps2         [PE]
      agg = max(agg, ps2 + b2)                   [DVE scalar_tensor_tensor]
    every DRAIN_EVERY-th tile instead drains ps2 on ACT
    (t2 = relu(ps2 + b2) -> bf16) and does a cheap 2x-mode bf16 max on
    DVE, balancing the ACT/DVE load (DVE's fused fp32-from-PSUM op is
    the more expensive of the two).
  - relu-before-max: since relu(max(x)) == max(relu(x)) and agg is
    initialized to 0 (>= 0 always), the max chain accumulates
    relu(h2 + b2) for free, so the tail consumes agg directly as the
    (bf16) input of the encoder matmul - no separate relu pass.
  - Tail per 512-node tile: enc = w3^T agg (b3 folded into b4'),
    r5 = relu(w4^T enc + b4'), dec-matmul w5 packed 4 tiles per PSUM
    group via tile_position col groups, tanh(+b5) -> [99, 512] fp32,
    DMA'd out as 4 dense [3, 512] strips. pos + 0.1*tanh is applied on
    the host during unpacking.
  - Nodes with no edges would aggregate garbage from pad slots; they are
    patched on the host with the closed-form constant output (their row
    is independent of x). With E/N = 16 there are essentially none.
"""
import sys

sys.path.insert(0, '/opt/trn_rl_repo')

import numpy as np
import ml_dtypes

import concourse.bass as bass
import concourse.tile as tile
from concourse import bacc, mybir
from concourse import bass_utils

F32 = mybir.dt.float32
BF16 = mybir.dt.bfloat16
BF = ml_dtypes.bfloat16

N_CORES = 8
TILE_W = 1024      # edge tile width (2 psum banks)
MM_W = 512         # max matmul moving free dim (1 psum bank)
CHUNK = 8192       # stream DMA chunk (cols); ring of N_BUFS buffers
N_BUFS = 3         # stream ring depth
NODE_W = 512       # tail node-tile width
GROUP = 4          # node tiles packed per psum group in the tail
WARMUP_MM = 14     # gapless matmul chain to ramp the PE p-state
DRAIN_EVERY = 6    # every Nth edge tile drains ps2 on ACT instead of DVE


def make_schedule(deg, n_nodes):
    """Common (all-cores) edge/tail tiling from the global degree array."""
    nodes_sorted = np.argsort(-deg, kind='stable')
    deg_sorted = deg[nodes_sorted]
    d_max = int(deg_sorted[0]) if len(deg_sorted) else 0
    M = np.searchsorted(-deg_sorted, -(np.arange(d_max) + 1), side='right')
    NC = n_nodes // N_CORES
    N_r = -(-M // N_CORES)              # ceil(M_r/8): common rank width
    T_r = -(-N_r // TILE_W)
    L = int((T_r * TILE_W).sum())
    rank_off = np.zeros(d_max + 1, np.int64)
    np.cumsum(T_r * TILE_W, out=rank_off[1:])
    etiles = []
    for r in range(d_max):
        w_left = int(N_r[r])
        for t in range(int(T_r[r])):
            w = min(TILE_W, w_left - t * TILE_W)
            etiles.append((int(rank_off[r]) + t * TILE_W, t * TILE_W, w))
    n_ntiles = -(-NC // NODE_W)
    n_groups = -(-n_ntiles // GROUP)
    return dict(nodes_sorted=nodes_sorted, deg_sorted=deg_sorted, d_max=d_max,
                NC=NC, N_r=N_r, T_r=T_r, L=L, rank_off=rank_off, etiles=etiles,
                n_ntiles=n_ntiles, n_groups=n_groups)


def build_nc(sched):
    NC, L = sched['NC'], sched['L']
    etiles = sched['etiles']
    n_ntiles, n_groups = sched['n_ntiles'], sched['n_groups']
    GPC = n_groups * NODE_W

    nc = bacc.Bacc("TRN2", target_bir_lowering=False, debug=False,
                   enable_asserts=False, num_devices=N_CORES)

    n_full = min(N_BUFS, -(-L // CHUNK))
    xs_d = nc.dram_tensor("xs", [6, L], BF16, kind="ExternalInput").ap()
    # first N_BUFS chunks shipped full-height: rows 0-5 stream data, rows
    # 6-127 zeros. The zeros land via DMA so no memset ever gates mm1,
    # and they persist across the ring (later chunk DMAs only rewrite
    # rows 0-5).
    xz_d = nc.dram_tensor("xz", [128, n_full * CHUNK], BF16,
                          kind="ExternalInput").ap()
    w1m_d = nc.dram_tensor("w1m", [128, 128], BF16, kind="ExternalInput").ap()
    w2_d = nc.dram_tensor("w2", [128, 128], BF16, kind="ExternalInput").ap()
    w3_d = nc.dram_tensor("w3", [128, 128], BF16, kind="ExternalInput").ap()
    w4_d = nc.dram_tensor("w4", [128, 128], BF16, kind="ExternalInput").ap()
    w5_d = nc.dram_tensor("w5", [128, 3], BF16, kind="ExternalInput").ap()
    b1_d = nc.dram_tensor("b1", [128, 1], F32, kind="ExternalInput").ap()
    b2_d = nc.dram_tensor("b2", [128, 1], F32, kind="ExternalInput").ap()
    b4p_d = nc.dram_tensor("b4p", [128, 1], F32, kind="ExternalInput").ap()
    b5pk_d = nc.dram_tensor("b5pk", [99, 1], F32, kind="ExternalInput").ap()
    out_d = nc.dram_tensor("outpk", [12, GPC], F32, kind="ExternalOutput").ap()

    RELU = mybir.ActivationFunctionType.Relu
    TANH = mybir.ActivationFunctionType.Tanh
    COPY = mybir.ActivationFunctionType.Copy
    ADD = mybir.AluOpType.add
    MAX = mybir.AluOpType.max

    with tile.TileContext(nc) as tc:
        with (
            tc.tile_pool(name="const", bufs=1) as cp,
            tc.tile_pool(name="aggp", bufs=1) as aggp,
            tc.tile_pool(name="stream", bufs=1) as sp,
            tc.tile_pool(name="work", bufs=4) as wp,
        ):
            # constants needed early
            w2_s = cp.tile([128, 128], BF16)
            nc.sync.dma_start(w2_s[:], w2_d[:])
            w1m_s = cp.tile([128, 128], BF16)
            nc.sync.dma_start(w1m_s[:], w1m_d[:])
            b1_s = cp.tile([128, 1], F32)
            nc.sync.dma_start(b1_s[:], b1_d[:])
            b2_s = cp.tile([128, 1], F32)
            nc.sync.dma_start(b2_s[:], b2_d[:])

            # PE warm-up needs this first on the vector queue
            warm_rhs = wp.tile([128, 512], BF16, tag="warmrhs")
            nc.vector.memset(warm_rhs[:], 0.0)

            # Stream chunk ring [128, CHUNK] x N_BUFS: rows 0-5 carry the
            # DMA'd [xi;xj] stream; rows 6-127 are zero so mm1 contracts
            # over K=128 with a zero-padded w1m. K=6 matmuls starve the
            # PE_HAM activity monitor (only 6 of 128 rows active) and pin
            # the PE at its 1.2 GHz throttled state - measured 454us of
            # K=4/8 with K=6, vs warm at K=128. The zeros arrive with the
            # first N_BUFS chunk DMAs (full-height, from xz) - a startup
            # PE gap re-throttles the HAM and a throttled stream never
            # recovers (measured 78-82us stuck at K=4/8), so nothing slow
            # may gate the first mm1.
            n_chunks = -(-L // CHUNK)
            ch_bufs = []
            for bi in range(N_BUFS):
                chb = sp.tile([128, CHUNK], BF16, tag=f"xs{bi}")
                ch_bufs.append(chb)

            # agg (bf16, init 0): relu-before-max makes 0 the identity.
            # Split the memset so the first columns are ready when the
            # first max lands (~17us).
            agg = aggp.tile([128, NC], BF16)
            A_SPLIT = min(4096, NC)
            nc.vector.memset(agg[:, :A_SPLIT], 0.0)
            if A_SPLIT < NC:
                nc.gpsimd.memset(agg[:, A_SPLIT:], 0.0)

            chunk_tiles = {}

            def emit_chunk_dma(ci):
                cw = min(CHUNK, L - ci * CHUNK)
                ch = ch_bufs[ci % N_BUFS]
                if ci < n_full:
                    nc.sync.dma_start(
                        ch[:, :cw], xz_d[:, ci * CHUNK: ci * CHUNK + cw])
                else:
                    nc.sync.dma_start(
                        ch[:6, :cw], xs_d[:, ci * CHUNK: ci * CHUNK + cw])
                chunk_tiles[ci] = ch

            for ci in range(min(2, n_chunks)):
                emit_chunk_dma(ci)

            # PE warm-up: gapless back-to-back matmul chain in its own
            # psum scope; the p-state ramp needs >3us of uninterrupted PE
            # execution.
            with tc.tile_pool(name="psW", bufs=4, space="PSUM") as pW:
                for i in range(WARMUP_MM):
                    wps = pW.tile([128, 512], F32, tag="warm")
                    nc.tensor.matmul(wps[:], w2_s[:], warm_rhs[:],
                                     start=True, stop=True)

            with (
                tc.tile_pool(name="psA", bufs=2, space="PSUM") as pA,
                tc.tile_pool(name="psB", bufs=2, space="PSUM") as pB,
            ):
                for ti, (so, c0, W) in enumerate(etiles):
                    ci, off = so // CHUNK, so % CHUNK
                    # prefetch 2 chunks ahead; emitting the DMA here (not
                    # up-front) keeps the WAR edge on the ring buffer
                    # behind this chunk's readers in program order
                    if ci + 2 < n_chunks and ci + 2 not in chunk_tiles:
                        emit_chunk_dma(ci + 2)
                    if ci not in chunk_tiles:
                        emit_chunk_dma(ci)
                    ch = chunk_tiles[ci]
                    ps1 = pA.tile([128, TILE_W], F32, tag="p1")
                    for h in range(0, W, MM_W):
                        w = min(MM_W, W - h)
                        nc.tensor.matmul(ps1[:, h:h + w], w1m_s[:],
                                         ch[:, off + h: off + h + w],
                                         start=True, stop=True)
                    h1 = wp.tile([128, TILE_W], BF16, tag="h1")
                    nc.scalar.activation(h1[:, :W], ps1[:, :W], RELU,
                                         bias=b1_s[:, 0:1])
                    ps2 = pB.tile([128, TILE_W], F32, tag="p2")
                    for h in range(0, W, MM_W):
                        w = min(MM_W, W - h)
                        nc.tensor.matmul(ps2[:, h:h + w], w2_s[:],
                                         h1[:, h:h + w], start=True, stop=True)
                    if ti % DRAIN_EVERY == DRAIN_EVERY - 1:
                        # balance: drain on ACT, cheap bf16 2x max on DVE
                        t2 = wp.tile([128, TILE_W], BF16, tag="t2")
                        nc.scalar.activation(t2[:, :W], ps2[:, :W], RELU,
                                             bias=b2_s[:, 0:1])
                        nc.vector.tensor_tensor(
                            out=agg[:, c0:c0 + W], in0=t2[:, :W],
                            in1=agg[:, c0:c0 + W], op=MAX)
                    else:
                        # fused add-b2 + max straight from PSUM
                        nc.vector.scalar_tensor_tensor(
                            out=agg[:, c0:c0 + W], in0=ps2[:, :W],
                            scalar=b2_s[:, 0:1], in1=agg[:, c0:c0 + W],
                            op0=ADD, op1=MAX)

            # tail constants (issued late so they don't delay the stream)
            w3_s = cp.tile([128, 128], BF16)
            nc.sync.dma_start(w3_s[:], w3_d[:])
            w4_s = cp.tile([128, 128], BF16)
            nc.sync.dma_start(w4_s[:], w4_d[:])
            w5_s = cp.tile([128, 3], BF16)
            nc.sync.dma_start(w5_s[:], w5_d[:])
            b4p_s = cp.tile([128, 1], F32)
            nc.sync.dma_start(b4p_s[:], b4p_d[:])
            b5pk_s = cp.tile([99, 1], F32)
            nc.sync.dma_start(b5pk_s[:], b5pk_d[:])

            with (
                tc.tile_pool(name="psT", bufs=2, space="PSUM") as pT,
                tc.tile_pool(name="psG", bufs=2, space="PSUM") as pG,
            ):
                for g in range(n_groups):
                    ps5 = pG.tile([99, NODE_W], F32, tag="p5")
                    for j in range(GROUP):
                        t = g * GROUP + j
                        if t >= n_ntiles:
                            # mm5 below only writes rows it owns; zero the
                            # missing strips so the out DMA reads zeros
                            if g == n_groups - 1:
                                nc.vector.memset(ps5[32 * j:32 * j + 3, :], 0.0)
                            continue
                        c0 = t * NODE_W
                        W = min(NODE_W, NC - c0)
                        # enc (sans b3): agg is already relu(max(...)+b2)
                        ps3 = pT.tile([128, NODE_W], F32, tag="p3")
                        nc.tensor.matmul(ps3[:, :W], w3_s[:], agg[:, c0:c0 + W],
                                         start=True, stop=True)
                        e4 = wp.tile([128, NODE_W], BF16, tag="e4")
                        if j % 2 == 0:
                            nc.scalar.activation(e4[:, :W], ps3[:, :W], COPY)
                        else:
                            nc.vector.tensor_copy(e4[:, :W], ps3[:, :W])
                        ps4 = pT.tile([128, NODE_W], F32, tag="p4")
                        nc.tensor.matmul(ps4[:, :W], w4_s[:], e4[:, :W],
                                         start=True, stop=True)
                        r5 = wp.tile([128, NODE_W], BF16, tag="r5")
                        if j % 2 == 0:
                            nc.vector.tensor_scalar(
                                out=r5[:, :W], in0=ps4[:, :W],
                                scalar1=b4p_s[:, 0:1], scalar2=0.0,
                                op0=ADD, op1=MAX)
                        else:
                            nc.scalar.activation(r5[:, :W], ps4[:, :W], RELU,
                                                 bias=b4p_s[:, 0:1])
                        if W < NODE_W:
                            nc.vector.memset(ps5[32 * j:32 * j + 3, W:], 0.0)
                        nc.tensor.matmul(ps5[32 * j:32 * j + 3, :W], w5_s[:],
                                         r5[:, :W], start=True, stop=True,
                                         tile_position=(0, 32 * j))
                    s_t = wp.tile([99, NODE_W], F32, tag="s")
                    nc.scalar.activation(s_t[:], ps5[:], TANH,
                                         bias=b5pk_s[:, 0:1])
                    gc = g * NODE_W
                    for j in range(GROUP):
                        nc.sync.dma_start(out_d[3 * j:3 * j + 3, gc:gc + NODE_W],
                                          s_t[32 * j:32 * j + 3, :])
    nc.compile()
    return nc


def make_inputs(x, pos, w1, b1, w2, b2, w3, b3, w4, b4, w5, b5,
                src, dst, sched):
    n_nodes = x.shape[0]
    E = src.shape[0]
    L, d_max = sched['L'], sched['d_max']
    N_r, rank_off = sched['N_r'], sched['rank_off']
    nodes_sorted = sched['nodes_sorted']

    order = np.argsort(dst, kind='stable')
    src_sorted = src[order]
    deg = np.bincount(dst, minlength=n_nodes)
    starts = np.zeros(n_nodes + 1, np.int64)
    np.cumsum(deg, out=starts[1:])

    # msg @ w1 = [xi ; xj-xi] @ w1 = [xi ; xj] @ [[w1a-w1b]; [w1b]]
    w1a, w1b = w1[:3], w1[3:]
    w1m = np.zeros((128, 128), np.float32)
    w1m[:6] = np.vstack([w1a - w1b, w1b])
    w1m = w1m.astype(BF)
    b4p = (b3 @ w4 + b4).astype(np.float32).reshape(128, 1)   # fold b3
    b5pk = np.zeros((99, 1), np.float32)
    for j in range(GROUP):
        b5pk[32 * j:32 * j + 3, 0] = b5

    common = dict(
        w1m=w1m, w2=w2.astype(BF), w3=w3.astype(BF), w4=w4.astype(BF),
        w5=w5.astype(BF), b1=b1.reshape(128, 1).astype(np.float32),
        b2=b2.reshape(128, 1).astype(np.float32), b4p=b4p, b5pk=b5pk)

    slot_pos = np.zeros(L, np.int64)
    for r in range(d_max):
        w = int(N_r[r])
        o = int(rank_off[r])
        slot_pos[o:o + w] = np.arange(w)

    in_maps = []
    for c in range(N_CORES):
        loc_nodes = nodes_sorted[c::N_CORES]
        loc_deg = deg[loc_nodes]
        loc_start = starts[loc_nodes]
        slot_src = np.zeros(L, np.int64)
        for r in range(d_max):
            w = int(N_r[r])
            o = int(rank_off[r])
            has = loc_deg[:w] > r
            # pad slots duplicate the node's first edge (max-idempotent);
            # deg-0 nodes gather garbage and are patched on the host
            idx = np.where(has, loc_start[:w] + r, loc_start[:w])
            np.minimum(idx, E - 1, out=idx)
            slot_src[o:o + w] = src_sorted[idx]
        xi_loc = x[loc_nodes]
        xs = np.empty((6, L), BF)
        xs[0:3] = xi_loc[slot_pos].T.astype(BF)
        xs[3:6] = x[slot_src].T.astype(BF)
        n_chunks = -(-L // CHUNK)
        n_full = min(N_BUFS, n_chunks)
        xz = np.zeros((128, n_full * CHUNK), BF)
        zw = min(L, n_full * CHUNK)
        xz[:6, :zw] = xs[:, :zw]
        in_maps.append(dict(xs=xs, xz=xz, **common))
    return in_maps


def unpack_outputs(results, sched, pos, deg, w3, b3, w4, b4, w5, b5):
    NC = sched['NC']
    nodes_sorted = sched['nodes_sorted']
    n_groups = sched['n_groups']
    n = len(nodes_sorted)
    out_full = np.zeros((n, 3), np.float32)
    for c in range(N_CORES):
        outpk = results[c]['outpk'].reshape(12, n_groups, NODE_W)
        tiles = np.zeros((3, n_groups * GROUP, NODE_W), np.float32)
        for j in range(GROUP):
            tiles[:, j::GROUP, :] = outpk[3 * j:3 * j + 3]
        tanh_t = tiles.reshape(3, -1)[:, :NC]
        loc = nodes_sorted[c::N_CORES]
        out_full[loc] = pos[loc] + 0.1 * tanh_t.T
    deg0 = deg == 0
    if deg0.any():
        # closed form for isolated nodes: agg = 0 -> enc = b3
        enc0 = b3
        dec0 = np.maximum(enc0 @ w4 + b4, 0.0) @ w5 + b5
        out_full[deg0] = pos[deg0] + 0.1 * np.tanh(dec0)
    return out_full


def run(inputs, trace=False, tmpdir=None):
    x = np.asarray(inputs['x'], np.float32)
    pos = np.asarray(inputs['pos'], np.float32)
    ei = np.asarray(inputs['edge_index'])
    src = ei[0].astype(np.int64)
    dst = ei[1].astype(np.int64)
    deg = np.bincount(dst, minlength=x.shape[0])
    sched = make_schedule(deg, x.shape[0])
    nc = build_nc(sched)
    args = [np.asarray(inputs[k], np.float32) for k in
            ('w1', 'b1', 'w2', 'b2', 'w3', 'b3', 'w4', 'b4', 'w5', 'b5')]
    in_maps = make_inputs(x, pos, *args, src, dst, sched)
    res = bass_utils.run_bass_kernel_spmd(
        nc, in_maps, core_ids=list(range(N_CORES)), trace=trace, tmpdir=tmpdir)
    w3_, b3_, w4_, b4_, w5_, b5_ = args[4:]
    out = unpack_outputs(res.results, sched, pos, deg,
                         w3_, b3_, w4_, b4_, w5_, b5_)
    return out, res


def kernel(**inputs):
    out, _ = run(inputs, trace=False)
    return out
